# revision 45
# baseline (speedup 1.0000x reference)
"""Trainium2 Bass kernel for nn_NeRF_MLP_Compose (MoE-routed NeRF MLP).

Strategy:
  - Host-side MoE dispatch (the sharding step): rows are permuted so each of
    the 8 cores receives a fixed-capacity, expert-contiguous block of rows
    (4 experts x 2176 rows, padded).  Each core then runs a dense per-expert
    MLP over its rows; outputs are inverse-permuted on the host.
  - All math (x normalize, positional encoding, 5 matmul layers, residuals,
    final division) runs on device.
  - Device layout: activations transposed (features on partitions, rows on
    the free dimension).  Positional encoding: theta built by a small
    "selection matmul" (freqs folded into the selection matrix), range
    reduction via DVE mod ops, ACT Sin.
  - Matmul weights and hidden activations in fp16 (1 PE pass/row vs 2 for
    f32r); the angle path (t5 matmul + range reduction) stays fp32 for
    phase accuracy at the high positional-encoding frequencies.
"""
import sys
for _p in ("/opt/trn_rl_repo", "/root/.axon_site/_ro/trn_rl_repo"):
    if _p not in sys.path:
        sys.path.insert(0, _p)

import numpy as np

N = 65536
E = 4            # experts
NCORE = 8
CAP = 2176       # rows per expert per core (17 * 128); global 17408 >= max cnt
ROWS_CORE = E * CAP          # 8704
NUM_FREQS = 10
HID = 256
DOUT = 64
NL = 4           # layers -> 3 residual blocks
TWO_PI = float(2 * np.pi)
TWO_PI_F32 = float(np.float32(2 * np.pi))
MAGIC_C = float(np.float32(1.5 * 2 ** 23))
CLAMP_HI = float(np.float32(0.5) - np.float32(2 ** -25))

_compiled = {}
RUN_KWARGS = {}    # test.py may set e.g. {"trace": True}
LAST_RESULT = []   # test.py reads the BassKernelResults appended here


def _freqs_f32():
    return (2.0 ** np.arange(NUM_FREQS, dtype=np.float32)) * np.float32(np.pi)


def _build_program():
    import concourse.bass as bass
    from concourse import bacc
    import concourse.mybir as mybir
    import concourse.tile as tile
    from concourse.masks import make_identity

    F32 = mybir.dt.float32
    F16 = mybir.dt.float16
    P = 128

    nc = bacc.Bacc("TRN2", target_bir_lowering=False, debug=False)

    # ---- DRAM I/O ----
    x_d = nc.dram_tensor("x_rows", [ROWS_CORE, 4], F32, kind="ExternalInput").ap()
    d_d = nc.dram_tensor("indim_rows", [ROWS_CORE], F32, kind="ExternalInput").ap()
    frq_d = nc.dram_tensor("freqs80", [80], F32, kind="ExternalInput").ap()
    phs_d = nc.dram_tensor("phase80", [80], F32, kind="ExternalInput").ap()
    w0f_d = nc.dram_tensor("w0f", [84, E, HID], F16, kind="ExternalInput").ap()
    wh_d = nc.dram_tensor("wh", [P, E, NL - 1, 2, HID], F16, kind="ExternalInput").ap()
    wo_d = nc.dram_tensor("wo", [P, E, 2, DOUT], F16, kind="ExternalInput").ap()
    b0_d = nc.dram_tensor("b0r", [P, E, 2], F32, kind="ExternalInput").ap()
    bh_d = nc.dram_tensor("bhr", [P, E, NL - 1, 2], F32, kind="ExternalInput").ap()
    bo_d = nc.dram_tensor("bor", [P, E], F32, kind="ExternalInput").ap()
    sc_d = nc.dram_tensor("scal12", [E * (NL - 1)], F32, kind="ExternalInput").ap()
    out_d = nc.dram_tensor("out_rows", [ROWS_CORE, DOUT], F16,
                           kind="ExternalOutput").ap()

    with tile.TileContext(nc) as tc:
        with tc.tile_pool(name="const", bufs=1) as cpool, \
             tc.tile_pool(name="work", bufs=3) as wpool, \
             tc.tile_pool(name="hbuf", bufs=3) as hpool, \
             tc.tile_pool(name="psA", bufs=1, space="PSUM") as psA, \
             tc.tile_pool(name="psB", bufs=2, space="PSUM") as psB:

            # ---- constants / weights into SBUF (once) ----
            ident16 = cpool.tile([P, P], F16)
            make_identity(nc, ident16)
            # signed freq / phase constants, replicated on all partitions;
            # feature f = j*20 + s*10 + i: freq = -+2^(i-1) (negated for cos:
            # cos(2 pi t) = sin(2 pi (0.25 - t))), phase = 0.25 on cos rows
            frep = cpool.tile([P, 4, 20], F32)
            nc.gpsimd.dma_start(
                out=frep, in_=bass.AP(tensor=frq_d.tensor, offset=0,
                                      ap=[[0, P], [20, 4], [1, 20]]))
            phrep = cpool.tile([P, 4, 20], F32)
            nc.gpsimd.dma_start(
                out=phrep, in_=bass.AP(tensor=phs_d.tensor, offset=0,
                                       ap=[[0, P], [20, 4], [1, 20]]))
            w0f = cpool.tile([84, E, HID], F16)
            nc.gpsimd.dma_start(out=w0f, in_=w0f_d)
            b0 = cpool.tile([P, E, 2], F32)
            nc.gpsimd.dma_start(out=b0, in_=b0_d)
            bh = cpool.tile([P, E, NL - 1, 2], F32)
            bo2 = cpool.tile([P, E], F32)
            scl = cpool.tile([P, E * (NL - 1)], F32)
            scl16 = cpool.tile([P, E * (NL - 1)], F16)
            wo = cpool.tile([P, E, 2, DOUT], F16)
            whs = {}
            for ee in range(E):
                for kk in range(NL - 1):
                    whs[ee, kk] = cpool.tile([P, 2, HID], F16,
                                             name=f"wh{ee}_{kk}")

            def issue_weight_dmas():
                # issued after the prologue preps so the input pipeline's
                # queue slots and DMA bandwidth come first; one tile per
                # (expert, layer) so the first matmuls only wait for their
                # own 260KB, not the whole 1.6MB
                engs = [nc.sync, nc.scalar, nc.gpsimd]
                for ee in range(E):
                    for kk in range(NL - 1):
                        engs[(ee * (NL - 1) + kk) % 3].dma_start(
                            out=whs[ee, kk],
                            in_=bass.AP(tensor=wh_d.tensor,
                                        offset=(ee * (NL - 1) + kk) * 2 * HID,
                                        ap=[[E * (NL - 1) * 2 * HID, P],
                                            [HID, 2], [1, HID]]))
                nc.gpsimd.dma_start(out=bh, in_=bh_d)
                nc.gpsimd.dma_start(out=bo2, in_=bo_d)
                nc.gpsimd.dma_start(
                    out=scl,
                    in_=bass.AP(tensor=sc_d.tensor, offset=0,
                                ap=[[0, P], [1, E * (NL - 1)]]))
                nc.gpsimd.tensor_copy(scl16, scl)
                nc.gpsimd.dma_start(out=wo, in_=wo_d)

            def prep_angle(r0, R, split=False):
                """Non-PE input side of one supertile: d load + row-major
                angle path on DVE/Pool/ACT.  split=True issues per-chunk
                pieces (lower latency to the first transpose, for the
                pipeline prologue); otherwise whole-supertile instructions."""
                c = R // P
                x_t = wpool.tile([P, 8, 4], F32, tag="x_t")
                d_t = wpool.tile([P, 8], F32, tag="d_t", bufs=4)
                rc0 = wpool.tile([P, 8], F32, tag="rc0")
                xn = wpool.tile([P, 8, 4], F32, tag="xn")
                uu = wpool.tile([P, 8, 4, 20], F32, tag="uu")
                kt = wpool.tile([P, 8, 80], F32, tag="kt")
                m0 = wpool.tile([P, 8, 80], F32, tag="m0")
                xer = wpool.tile([P, 8, 84], F16, tag="xer", bufs=4)
                pieces = [(ch, 1) for ch in range(c)] if split else [(0, c)]
                for p0, pc in pieces:
                    ps = slice(p0, p0 + pc)
                    nc.sync.dma_start(
                        out=x_t[:, ps, :],
                        in_=bass.AP(tensor=x_d.tensor, offset=(r0 + p0 * P) * 4,
                                    ap=[[4, P], [4 * P, pc], [1, 4]]))
                    # normalize: xn = x * (1/x3) (walrus has no divide ALU
                    # op), then restore x3
                    nc.vector.reciprocal(rc0[:, ps], x_t[:, ps, 3])
                    nc.vector.tensor_mul(
                        xn[:, ps, :], x_t[:, ps, :],
                        rc0[:, ps, None].to_broadcast((P, pc, 4)))
                    nc.vector.tensor_copy(xn[:, ps, 3], x_t[:, ps, 3])
                    # u = x'_j * (-+2^(i-1)) + phase, in turns (exact mult:
                    # power-of-two freqs).  k=round(u) via the fp32 magic-add
                    # trick; m0 = u - k in [-.5, .5].  mul/add on Pool.
                    nc.gpsimd.tensor_mul(
                        uu[:, ps, :, :],
                        xn[:, ps, :, None].to_broadcast((P, pc, 4, 20)),
                        frep[:, None, :, :].to_broadcast((P, pc, 4, 20)))
                    nc.gpsimd.tensor_add(
                        uu[:, ps, :, :], uu[:, ps, :, :],
                        phrep[:, None, :, :].to_broadcast((P, pc, 4, 20)))
                    nc.vector.tensor_scalar(
                        kt[:, ps, :],
                        uu[:, ps, :, :].rearrange("p c j k -> p c (j k)"),
                        MAGIC_C, MAGIC_C,
                        mybir.AluOpType.add, mybir.AluOpType.subtract)
                    nc.vector.scalar_tensor_tensor(
                        m0[:, ps, :], kt[:, ps, :], -1.0,
                        uu[:, ps, :, :].rearrange("p c j k -> p c (j k)"),
                        mybir.AluOpType.mult, mybir.AluOpType.add)
                    # row-major encoded features [sin(80), x'(4)], fp16
                    nc.scalar.activation(xer[:, ps, 0:80], m0[:, ps, :],
                                         mybir.ActivationFunctionType.Sin,
                                         bias=0.0, scale=TWO_PI_F32)
                    nc.gpsimd.tensor_copy(xer[:, ps, 80:84], xn[:, ps, :])
                nc.sync.dma_start(
                    out=d_t[:, :c],
                    in_=bass.AP(tensor=d_d.tensor, offset=r0,
                                ap=[[1, P], [P, c]]))
                return xer, d_t

            def xpose_batch(xer, xe84, R, half):
                """PE transpose of up to 4 chunks of xer into xe84; issued
                inside the previous supertile's MLP to fill PE stall slots."""
                c = R // P
                hc = min(4, c - half * 4)
                if hc <= 0:
                    return
                ps_xe = psA.tile([84, 4, P], F16, tag="xe", name=f"ps_xe{half}",
                                 bufs=2)
                for ch in range(hc):
                    nc.tensor.transpose(ps_xe[:, ch, :],
                                        xer[:, half * 4 + ch, :], ident16)
                nc.scalar.copy(
                    xe84[:, half * 512:half * 512 + hc * P],
                    ps_xe[:, :hc, :].rearrange("p c q -> p (c q)"))

            def mlp_super(e, r0, R, xe84, d_t, fillers=()):
                """MLP + output side of one supertile; chunk-inner matmul
                ordering so consecutive matmuls share the stationary weights
                (the post-build pass below drops the repeated ldweights)."""
                chunks = [(0, 512), (512, 512)] if R == 1024 else [(0, R)]
                ct = R // P

                # layer 0: h0 = relu(W0f^T xe84 + b0)
                ps_zs = {(i, mb): psB.tile([P, 512], F32, tag="z",
                                           name=f"ps_z{i}_{mb}", bufs=4)
                         for mb in range(2) for i in range(len(chunks))}
                for mb in range(2):
                    for i, (off, Rc) in enumerate(chunks):
                        nc.tensor.matmul(ps_zs[i, mb][:, :Rc],
                                         w0f[:, e, mb * P:(mb + 1) * P],
                                         xe84[:, off:off + Rc],
                                         start=True, stop=True)
                # relus on ACT, half 0 (needed by the next layer's kb=0
                # matmuls) first, so the PE restarts as early as possible
                h = hpool.tile([P, 2, 1024], F16, tag="h", bufs=4)
                for mb in range(2):
                    for i, (off, Rc) in enumerate(chunks):
                        nc.scalar.activation(
                            h[:, mb, off:off + Rc], ps_zs[i, mb][:, :Rc],
                            mybir.ActivationFunctionType.Relu,
                            bias=b0[:, e, mb:mb + 1], scale=1.0)
                if len(fillers) > 0:
                    fillers[0]()

                # hidden residual layers
                for k in range(NL - 1):
                    ps_zk = {(i, mb): psB.tile([P, 512], F32, tag="z",
                                               name=f"ps_zk{i}_{mb}", bufs=4)
                             for mb in range(2) for i in range(len(chunks))}
                    for mb in range(2):
                        for kb in range(2):
                            for i, (off, Rc) in enumerate(chunks):
                                nc.tensor.matmul(
                                    ps_zk[i, mb][:, :Rc],
                                    whs[e, k][:, kb, mb * P:(mb + 1) * P],
                                    h[:, kb, off:off + Rc],
                                    start=(kb == 0), stop=(kb == 1))
                    # relu on ACT; residual pieces on DVE in the order the
                    # next layer's matmuls consume them (half 0 first)
                    t = hpool.tile([P, 2, 1024], F16, tag="t", bufs=4)
                    h_new = hpool.tile([P, 2, 1024], F16, tag="h", bufs=4)
                    idx = e * (NL - 1) + k
                    for mb in range(2):
                        for i, (off, Rc) in enumerate(chunks):
                            nc.scalar.activation(
                                t[:, mb, off:off + Rc], ps_zk[i, mb][:, :Rc],
                                mybir.ActivationFunctionType.Relu,
                                bias=bh[:, e, k, mb:mb + 1], scale=1.0)
                        for i, (off, Rc) in enumerate(chunks):
                            nc.vector.scalar_tensor_tensor(
                                h_new[:, mb, off:off + Rc],
                                t[:, mb, off:off + Rc],
                                scl16[:, idx:idx + 1],
                                h[:, mb, off:off + Rc],
                                mybir.AluOpType.mult, mybir.AluOpType.add)
                    if len(fillers) > k + 1:
                        fillers[k + 1]()
                    h = h_new

                # output layer: o = Wout^T h3 + bout.  The two row chunks
                # land in PE column halves (tile_position), so one [128,128]
                # transpose covers 128 rows of BOTH chunks at once.
                ps_o = psA.tile([P, 512], F32, tag="o")
                for kb in range(2):
                    for i, (off, Rc) in enumerate(chunks):
                        nc.tensor.matmul(ps_o[i * DOUT:(i + 1) * DOUT, :Rc],
                                         wo[:, e, kb, :], h[:, kb, off:off + Rc],
                                         start=(kb == 0), stop=(kb == 1))
                nh = len(chunks)
                oT = wpool.tile([P, 512], F16, tag="oT")
                nc.scalar.activation(oT[:nh * DOUT, :chunks[0][1]],
                                     ps_o[:nh * DOUT, :chunks[0][1]],
                                     mybir.ActivationFunctionType.Identity,
                                     bias=bo2[:nh * DOUT, e:e + 1], scale=1.0)

                # transpose back to rows, divide by in_dim, store
                cc = chunks[0][1] // P   # 128-row groups per chunk
                ps_t = psA.tile([P, 4, P], F16, tag="t")
                for ch in range(cc):
                    nc.tensor.transpose(ps_t[:, ch, :nh * DOUT],
                                        oT[:nh * DOUT, ch * P:(ch + 1) * P],
                                        ident16[:nh * DOUT, :nh * DOUT])
                rid = wpool.tile([P, 8], F32, tag="rid")
                nc.vector.reciprocal(rid[:, :ct], d_t[:, :ct])
                o_rows = wpool.tile([P, 2, 4, DOUT], F16, tag="o_rows")
                nc.vector.tensor_mul(
                    o_rows[:, :nh, :cc, :],
                    ps_t[:, :cc, :nh * DOUT].rearrange(
                        "p c (h q) -> p h c q", h=nh),
                    bass.AP(tensor=rid.tensor, offset=rid.offset,
                            ap=[rid.ap[0], [cc, nh], [1, cc], [0, DOUT]]))
                nc.sync.dma_start(
                    out=bass.AP(tensor=out_d.tensor, offset=r0 * DOUT,
                                ap=[[DOUT, P], [P * DOUT, nh * cc], [1, DOUT]]),
                    in_=o_rows[:, :nh, :cc, :])

            # software pipeline: the angle path runs 2 supertiles ahead on
            # DVE/Pool/ACT; its PE transposes are issued as fillers inside
            # the previous supertile's MLP, landing in the PE's natural
            # dependency-stall slots after each layer's matmuls
            stiles = []
            for e in range(E):
                r0 = e * CAP
                for R in [1024, 1024, 128]:
                    stiles.append((e, r0, R))
                    r0 += R
            S = len(stiles)
            angles = {}
            xe84s = {}

            def angle(j, split=False):
                angles[j] = prep_angle(stiles[j][1], stiles[j][2], split)

            def make_fillers(j):
                if j >= S:
                    return []
                Rj = stiles[j][2]
                xe84s[j] = wpool.tile([84, 1024], F16, tag="xe84",
                                      name=f"xe84_{j}")
                xer_j = angles[j][0]
                return [
                    (lambda half=half, xj=xer_j, ej=xe84s[j], Rj=Rj:
                     xpose_batch(xj, ej, Rj, half))
                    for half in range(2 if Rj == 1024 else 1)]

            angle(0)
            angle(1)
            angle(2)
            for f in make_fillers(0):
                f()
            issue_weight_dmas()
            for i, (e, r0, R) in enumerate(stiles):
                if i + 3 < S:
                    angle(i + 3)
                fillers = make_fillers(i + 1)
                mlp_super(e, r0, R, xe84s[i], angles[i][1], fillers)
                del angles[i], xe84s[i]

    _dedup_ldweights(nc, mybir)
    nc.compile()
    return nc


def _dedup_ldweights(nc, mybir):
    """Drop an InstLdweights whose stationary AP equals the previous one on
    the PE queue with only matmuls in between: the PE keeps its loaded
    weights across matmuls, so the reload is redundant.  Sync deps of the
    dropped load are moved to the next PE instruction."""
    removed = 0
    for f in nc.m.functions:
        for b in f.blocks:
            new_insts = []
            last_lw = None
            pending = []
            for inst in b.instructions:
                if isinstance(inst, mybir.InstLdweights):
                    k = (str(inst.ins[0]), str(inst.tile_position))
                    if last_lw is not None and last_lw == k:
                        pending.append(inst)
                        removed += 1
                        continue
                    last_lw = k
                elif isinstance(inst, mybir.InstMatmult):
                    pass
                elif getattr(inst, "engine", None) == mybir.EngineType.PE:
                    last_lw = None
                if pending and getattr(inst, "engine", None) == mybir.EngineType.PE:
                    for dl in pending:
                        inst.add_sync_dependencies_from(
                            dl.take_sync_dependencies())
                    pending = []
                new_insts.append(inst)
            assert not pending, "dropped ldweights with no PE successor"
            b.instructions[:] = new_insts
    return removed


def _get_program():
    if "nc" not in _compiled:
        _compiled["nc"] = _build_program()
    return _compiled["nc"]


def _prep_weights(W0, b0, Wh, bh, scal, Wout, bout):
    """Host-side layout transforms (permutation / reshape / replication only)."""
    # device feature order: f = j*20 + s*10 + i for f < 80 (j: input dim,
    # s: 0=sin 1=cos, i: freq), then x'_{0..3} at 80..83.
    # reference xe column order: 4 + i*8 + j*2 + s (x' at 0..3).
    # freq = -+2^(i-1) in turns (negated for cos: cos(2 pi t) =
    # sin(2 pi (0.25 - t))); phase = 0.25 on cos features.
    freqs80 = np.zeros(80, np.float32)
    phase80 = np.zeros(80, np.float32)
    perm = np.zeros(84, np.int64)
    for j in range(4):
        for s in range(2):
            for i in range(NUM_FREQS):
                f = j * 20 + s * 10 + i
                freqs80[f] = np.float32((-1.0 if s else 1.0) * 2.0 ** (i - 1))
                phase80[f] = 0.25 if s else 0.0
                perm[f] = 4 + i * 8 + j * 2 + s
    perm[80:84] = np.arange(4)
    w0f = np.ascontiguousarray(
        W0[:, perm, :].transpose(1, 0, 2)).astype(np.float16)    # [84,E,H]
    wh = np.ascontiguousarray(
        Wh.reshape(E, NL - 1, 2, 128, HID).transpose(3, 0, 1, 2, 4)
    ).astype(np.float16)                                          # [128,E,3,2,H]
    wo = np.ascontiguousarray(
        Wout.reshape(E, 2, 128, DOUT).transpose(2, 0, 1, 3)
    ).astype(np.float16)                                          # [128,E,2,Do]
    b0r = np.ascontiguousarray(b0.reshape(E, 2, 128).transpose(2, 0, 1))
    bhr = np.ascontiguousarray(
        bh.reshape(E, NL - 1, 2, 128).transpose(3, 0, 1, 2))
    bor = np.ascontiguousarray(
        np.concatenate([bout, bout], axis=1).transpose(1, 0))    # [128,E]
    sc12 = np.ascontiguousarray(scal.reshape(-1))
    return dict(freqs80=freqs80, phase80=phase80, w0f=w0f, wh=wh, wo=wo,
                b0r=b0r, bhr=bhr, bor=bor, scal12=sc12)


def kernel(x, in_dim, layer_id, W0, b0, Wh, bh, scal, Wout, bout):
    from concourse.bass_utils import run_bass_kernel_spmd

    x = np.asarray(x, np.float32)
    in_dim = np.asarray(in_dim, np.float32)
    layer_id = np.asarray(layer_id)

    # ---- dispatch: per-expert row indices, split evenly across cores ----
    PADIDX = N
    x_aug = np.vstack([x, np.ones((1, 4), np.float32)])
    d_aug = np.concatenate([in_dim, np.ones(1, np.float32)])
    perms = np.full((NCORE, ROWS_CORE), PADIDX, np.int64)
    overflow = []
    for e in range(E):
        idx = np.flatnonzero(layer_id == e)
        if len(idx) > NCORE * CAP:
            overflow.append(idx[NCORE * CAP:])
            idx = idx[:NCORE * CAP]
        for c, part in enumerate(np.array_split(idx, NCORE)):
            perms[c, e * CAP:e * CAP + len(part)] = part

    wmaps = _prep_weights(np.asarray(W0, np.float32), np.asarray(b0, np.float32),
                          np.asarray(Wh, np.float32), np.asarray(bh, np.float32),
                          np.asarray(scal, np.float32),
                          np.asarray(Wout, np.float32),
                          np.asarray(bout, np.float32))

    in_maps = []
    for c in range(NCORE):
        p = perms[c]
        m = dict(wmaps)
        m["x_rows"] = np.ascontiguousarray(x_aug[p])
        m["indim_rows"] = np.ascontiguousarray(d_aug[p])
        in_maps.append(m)

    nc = _get_program()
    res = run_bass_kernel_spmd(nc, in_maps, core_ids=list(range(NCORE)),
                               **RUN_KWARGS)
    LAST_RESULT.clear()
    LAST_RESULT.append(res)

    out = np.zeros((N + 1, DOUT), np.float32)
    for c in range(NCORE):
        out[perms[c]] = res.results[c]["out_rows"].astype(np.float32)

    # pathological overflow fallback (never hit for the benchmark input)
    if overflow:
        ov = np.concatenate(overflow)
        out[ov] = _numpy_ref(x[ov], in_dim[ov], layer_id[ov], W0, b0, Wh, bh,
                             scal, Wout, bout)
    return out[:N]


def _numpy_ref(x, in_dim, layer_id, W0, b0, Wh, bh, scal, Wout, bout):
    x = np.concatenate([x[:, :3] / x[:, 3:4], x[:, 3:]], axis=1)
    freqs = _freqs_f32()
    ang = x[:, None, :] * freqs[None, :, None]
    sc = np.stack([np.sin(ang), np.cos(ang)], axis=-1)
    xe = np.concatenate([x, sc.reshape(x.shape[0], -1)], axis=1)
    out = np.zeros((x.shape[0], DOUT), np.float32)
    for e in range(E):
        m = layer_id == e
        if not m.any():
            continue
        h = np.maximum(xe[m] @ W0[e] + b0[e], 0.0)
        for k in range(NL - 1):
            h = scal[e, k] * np.maximum(h @ Wh[e, k] + bh[e, k], 0.0) + h
        out[m] = h @ Wout[e] + bout[e]
    return out / in_dim[:, None]
